# revision 8
# baseline (speedup 1.0000x reference)
import sys

sys.path.insert(0, "/opt/trn_rl_repo")

import ml_dtypes
import numpy as np

import concourse.bass as bass
import concourse.tile as tile
from concourse import bacc, mybir
from concourse.bass_utils import run_bass_kernel_spmd

N_CORES = 8
B, C, D = 4096, 10000, 512
CL = C // N_CORES          # classes per core (tensor-parallel shard)
KC = D // 128              # contraction chunks
MC = B // 128              # batch chunks of 128 rows
FCH = 1024                 # featT DMA chunk (columns) for load pipelining
N_SLICES = [(0, 512), (512, 512), (1024, 226)]

BF16 = ml_dtypes.bfloat16


def _build():
    nc = bacc.Bacc("TRN2", target_bir_lowering=False, debug=False,
                   num_devices=N_CORES)
    featT = nc.dram_tensor("featT", [KC, 128, B], mybir.dt.bfloat16,
                           kind="ExternalInput")
    wT = nc.dram_tensor("wT", [KC, 128, CL], mybir.dt.bfloat16,
                        kind="ExternalInput")
    w2b = nc.dram_tensor("w2b", [128, CL], mybir.dt.float32,
                         kind="ExternalInput")
    f2c = nc.dram_tensor("f2c", [128, MC], mybir.dt.float32,
                         kind="ExternalInput")
    out = nc.dram_tensor("out", [B, CL], mybir.dt.float16,
                         kind="ExternalOutput")
    fT, wTa, w2a, f2a, outa = (featT.ap(), wT.ap(), w2b.ap(), f2c.ap(),
                               out.ap())

    with tile.TileContext(nc) as tc:
        with (
            tc.tile_pool(name="ft", bufs=1) as ftp,
            tc.tile_pool(name="wt", bufs=1) as wtp,
            tc.tile_pool(name="cst", bufs=1) as cst,
            tc.tile_pool(name="ps", bufs=2, space="PSUM") as psp,
            tc.tile_pool(name="add", bufs=3) as addp,
            tc.tile_pool(name="exp", bufs=3) as expp,
        ):
            # loads ordered so the m=0 tile's deps land first:
            # wt n=0 slices -> first featT chunk -> rest of wt -> rest of ft
            wt = [wtp.tile([128, CL], mybir.dt.bfloat16, tag=f"wt{k}",
                           name=f"wt{k}") for k in range(KC)]
            ft = [ftp.tile([128, B], mybir.dt.bfloat16, tag=f"ft{k}",
                           name=f"ft{k}") for k in range(KC)]
            for k in range(KC):
                ns, nl = N_SLICES[0]
                nc.sync.dma_start(wt[k][:, ns:ns + nl], wTa[k][:, ns:ns + nl])
            for k in range(KC):
                nc.sync.dma_start(ft[k][:, 0:FCH], fT[k][:, 0:FCH])
            for ns, nl in N_SLICES[1:]:
                for k in range(KC):
                    nc.sync.dma_start(wt[k][:, ns:ns + nl],
                                      wTa[k][:, ns:ns + nl])
            w2t = cst.tile([128, CL], mybir.dt.float32, tag="w2")
            nc.sync.dma_start(w2t[:], w2a[:])
            f2t = cst.tile([128, MC], mybir.dt.float32, tag="f2")
            nc.sync.dma_start(f2t[:], f2a[:])
            for j in range(1, B // FCH):
                for k in range(KC):
                    nc.sync.dma_start(ft[k][:, bass.ts(j, FCH)],
                                      fT[k][:, bass.ts(j, FCH)])

            for m in range(MC):
                ps = psp.tile([128, CL], mybir.dt.float32, name="ps",
                              tag="ps")
                for k in range(KC):
                    lhsT = ft[k][:, bass.ts(m, 128)]
                    for ns, nl in N_SLICES:
                        nc.tensor.matmul(ps[:, ns:ns + nl], lhsT,
                                         wt[k][:, ns:ns + nl],
                                         start=(k == 0), stop=(k == KC - 1))
                t = addp.tile([128, CL], mybir.dt.float32)
                nc.vector.tensor_add(t[:], ps[:], w2t[:])
                o = expp.tile([128, CL], mybir.dt.float16)
                nc.scalar.activation(o[:], t[:],
                                     mybir.ActivationFunctionType.Exp,
                                     bias=f2t[:, m:m + 1], scale=-0.01)
                nc.sync.dma_start(outa[bass.ts(m, 128), :], o[:])
    nc.compile()
    return nc


_NC = None


def _prep(feat, weights):
    featT2 = np.ascontiguousarray(
        (feat.T * np.float32(-2.0)).astype(BF16)).reshape(KC, 128, B)
    f2 = np.einsum("bd,bd->b", feat.astype(np.float64),
                   feat.astype(np.float64))
    f2c = np.ascontiguousarray(
        (-0.01 * f2).reshape(MC, 128).T).astype(np.float32)
    in_maps = []
    for i in range(N_CORES):
        ws = weights[i * CL:(i + 1) * CL]
        wT = np.ascontiguousarray(ws.T.astype(BF16)).reshape(KC, 128, CL)
        w2 = np.einsum("cd,cd->c", ws.astype(np.float64),
                       ws.astype(np.float64)).astype(np.float32)
        w2b = np.ascontiguousarray(np.broadcast_to(w2[None, :], (128, CL)))
        in_maps.append({"featT": featT2, "wT": wT, "w2b": w2b, "f2c": f2c})
    return in_maps


def kernel(feat, label, weights):
    global _NC
    feat = np.ascontiguousarray(feat, dtype=np.float32)
    weights = np.ascontiguousarray(weights, dtype=np.float32)
    if _NC is None:
        _NC = _build()
    in_maps = _prep(feat, weights)
    res = run_bass_kernel_spmd(_NC, in_maps, core_ids=list(range(N_CORES)))
    full = np.concatenate([res.results[i]["out"].astype(np.float32)
                           for i in range(N_CORES)], axis=1)
    return (full, full, weights)
